# revision 1
# baseline (speedup 1.0000x reference)
"""Trainium2 Bass kernel for nn_AttLayer_9972914061697 (sparse_attention).

Reference computation (jax):
    q, k, v = split(x, 3, axis=-1)              # x: [B=4, T=4096, 3C=384]
    score   = einsum('btc,bsc->bts', k, q) / sqrt(C)
    out     = softmax(score, -1) @ v            # [B, T, C=128]

Sharding: 8 cores = 4 batches x 2 T-halves (data parallel, zero comm).
Each core holds the full q, v of its batch plus its 2048-row k chunk and
produces its 2048-row output chunk. q/k/v are shipped as bf16 (the matmul
compute dtype; identical numerics to an on-device cast), output is f32.

Per-core algorithm (matmuls bf16, accumulation f32):
  - q, k land transposed in SBUF via XBAR DMA-transpose ([C, T] layout)
  - S_T[s, t] = sum_c q[s,c] k[t,c] computed as qT_chunk.T @ kT (PSUM f32)
  - P_T = exp(S_T / sqrt(C)) via ScalarE, written bf16 to SBUF
  - out[t, 0:128] & rowsum[t] in one PSUM accumulation: P_T_chunk.T @ [v | 1]
    (ones column appended to v makes the softmax denominator an extra column)
  - out = out * 1/rowsum (VectorE reciprocal + per-partition scalar mul)

The s axis runs in groups of GSC 128-row chunks, software-pipelined:
group g's QK+exp is emitted before group g-1's PV so the ScalarE exp stream
never starves, and loads for later groups are issued alongside.
"""

import numpy as np
import ml_dtypes

import concourse.bass as bass
import concourse.tile as tile
from concourse import bacc, mybir
from concourse.bass_utils import run_bass_kernel_spmd

F32 = mybir.dt.float32
BF16 = mybir.dt.bfloat16

B = 4
T = 4096
C = 128
N_CORES = 8
TL = T // 2          # 2048 t-rows per core
NSC = T // 128       # 32 s-chunks
NTT = TL // 128      # 16 t-tiles
GSC = 4              # s-chunks per group (PV flush granularity)
NG = NSC // GSC      # 8 groups
GW = GSC * 128       # s-rows per group (512)
SCALE = 1.0 / float(np.sqrt(C))


def build_nc():
    nc = bacc.Bacc()
    q = nc.declare_dram_parameter("q", [T, C], BF16, isOutput=False)
    k = nc.declare_dram_parameter("k", [TL, C], BF16, isOutput=False)
    v = nc.declare_dram_parameter("v", [T, C], BF16, isOutput=False)
    out = nc.declare_dram_parameter("out", [TL, C], F32, isOutput=True)

    vw = v[:].rearrange("(n p) c -> p n c", p=128)    # [128, 32, 128]
    ov = out[:].rearrange("(n p) c -> p n c", p=128)  # [128, 16, 128]

    with tile.TileContext(nc) as tc:
        with (
            tc.tile_pool(name="const", bufs=1) as const_pool,
            tc.tile_pool(name="qkt", bufs=1) as qkt_pool,
            tc.tile_pool(name="vbuf", bufs=1) as v_pool,
            tc.tile_pool(name="pT", bufs=3) as pT_pool,
            tc.tile_pool(name="oacc", bufs=1) as oacc_pool,
            tc.tile_pool(name="ost", bufs=2) as ost_pool,
            tc.tile_pool(name="spsum", bufs=3, space="PSUM") as spsum,
            tc.tile_pool(name="opsum", bufs=2, space="PSUM") as opsum,
        ):
            qT = qkt_pool.tile([128, T], BF16, tag="qT")     # q transposed [c, s]
            kT = qkt_pool.tile([128, TL], BF16, tag="kT")    # k transposed [c, t]
            vv = v_pool.tile([128, NSC * (C + 1)], BF16)     # 32 x [128, 129] chunks
            vv3 = vv[:].rearrange("p (n c) -> p n c", c=C + 1)
            oacc = oacc_pool.tile([128, NTT * (C + 1)], F32)
            oacc3 = oacc[:].rearrange("p (n c) -> p n c", c=C + 1)
            rtile = const_pool.tile([128, NTT], F32, tag="recip")

            # ones column of every v chunk
            nc.vector.memset(vv3[:, :, C : C + 1], 1.0)

            # warm up the ACT exp table early so the ~2.7us table load
            # overlaps the prologue DMA instead of stalling the first score
            warm = const_pool.tile([128, 8], F32, tag="warm")
            nc.vector.memset(warm[:], 0.0)
            nc.scalar.activation(
                warm[:], warm[:], mybir.ActivationFunctionType.Exp, scale=1.0
            )

            import os as _os

            if _os.environ.get("KCFG_TAIL22"):
                GROUPS = [(i * 4, 4) for i in range(7)] + [(28, 2), (30, 2)]
            else:
                GROUPS = [(i * 4, 4) for i in range(8)]
            split_final = bool(_os.environ.get("KCFG_SPLIT"))

            def load_qT(s0, n, eng=None):
                (eng or nc.sync).dma_start(
                    out=qT[:, s0 * 128 : (s0 + n) * 128],
                    in_=q[s0 * 128 : (s0 + n) * 128, :],
                    transpose=True,
                )

            def load_v(s0, n, eng=None):
                (eng or nc.sync).dma_start(
                    out=vv3[:, s0 : s0 + n, 0:C],
                    in_=vw[:, s0 : s0 + n, :],
                )

            # prologue loads: all DMA stays on the Sync HWDGE — concurrent
            # XBAR transposes from two HWDGEs corrupt the shared XBAR, and
            # mixing plain DMA on the other HWDGE serializes on mode switches.
            load_qT(0, 4)
            for j in range(4):
                nc.sync.dma_start(
                    out=kT[:, j * 512 : (j + 1) * 512],
                    in_=k[j * 512 : (j + 1) * 512, :],
                    transpose=True,
                )
            load_qT(4, 4)
            load_v(0, 4)

            def qk_exp_group(g, pT):
                s0, gn = GROUPS[g]
                for lc in range(gn):
                    sc = s0 + lc
                    lhs = qT[:, sc * 128 : (sc + 1) * 128]
                    for h in range(2):  # two [128, 1024] halves of t
                        ps = spsum.tile([128, 1024], F32, tag="s")
                        for j in range(2):
                            t_off = h * 1024 + j * 512
                            nc.tensor.matmul(
                                ps[:, j * 512 : (j + 1) * 512],
                                lhs,
                                kT[:, t_off : t_off + 512],
                                start=True,
                                stop=True,
                            )
                        nc.scalar.activation(
                            pT[:, lc * TL + h * 1024 : lc * TL + (h + 1) * 1024],
                            ps[:],
                            mybir.ActivationFunctionType.Exp,
                            scale=SCALE,
                        )

            def pv_group(g, pT, final, lc0=0, nlc=None):
                s0, gn = GROUPS[g]
                s0 += lc0
                if nlc is None:
                    nlc = gn
                ost = None
                for tt2 in range(NTT // 2):  # pairs of t-tiles per PSUM bank
                    op = opsum.tile([128, 2 * (C + 1)], F32, tag="o")
                    for half in range(2):
                        tt = tt2 * 2 + half
                        for i in range(nlc):
                            lc = lc0 + i
                            nc.tensor.matmul(
                                op[:, half * (C + 1) : (half + 1) * (C + 1)],
                                pT[:, lc * TL + tt * 128 : lc * TL + (tt + 1) * 128],
                                vv3[:, s0 + i, :],
                                start=(i == 0),
                                stop=(i == nlc - 1),
                            )
                    dst = oacc[:, tt2 * 2 * (C + 1) : (tt2 + 1) * 2 * (C + 1)]
                    if g == 0 and lc0 == 0:
                        nc.vector.tensor_copy(dst, op[:])
                    else:
                        nc.vector.tensor_add(dst, dst, op[:])
                    if final:
                        # normalize + store as soon as each t-tile pair is done
                        if tt2 % 2 == 0:
                            ost = ost_pool.tile([128, 4, 128], F32, tag="ost")
                        for half in range(2):
                            tt = tt2 * 2 + half
                            nc.vector.reciprocal(
                                rtile[:, tt : tt + 1], oacc3[:, tt, C : C + 1]
                            )
                            nc.vector.tensor_scalar_mul(
                                ost[:, (tt2 % 2) * 2 + half, :],
                                oacc3[:, tt, 0:C],
                                rtile[:, tt : tt + 1],
                            )
                        if tt2 % 2 == 1:
                            tt0 = (tt2 - 1) * 2
                            nc.sync.dma_start(
                                out=ov[:, tt0 : tt0 + 4, :], in_=ost[:]
                            )

            # ---- software-pipelined main loop ----
            ngr = len(GROUPS)
            pT_tiles = {}
            for g in range(ngr):
                if g + 2 < ngr:
                    load_qT(*GROUPS[g + 2])
                if g + 1 < ngr:
                    load_v(*GROUPS[g + 1])
                pT_g = pT_pool.tile([128, GROUPS[g][1] * TL], BF16, tag="pT")
                pT_tiles[g] = pT_g
                qk_exp_group(g, pT_tiles[g])
                if g >= 1:
                    pv_group(g - 1, pT_tiles[g - 1], final=False)
                    del pT_tiles[g - 1]
            # final group's PV in two halves: the first half's matmuls only
            # need the first two chunks' exps, so they overlap the last exps
            gl, nl = ngr - 1, GROUPS[-1][1]
            if split_final and nl >= 2:
                pv_group(gl, pT_tiles[gl], final=False, lc0=0, nlc=nl // 2)
                pv_group(gl, pT_tiles[gl], final=True, lc0=nl // 2, nlc=nl - nl // 2)
            else:
                pv_group(gl, pT_tiles[gl], final=True)

    nc.finalize()
    return nc


_NC_CACHE = None


def make_in_maps(x: np.ndarray):
    xb = np.asarray(x, dtype=np.float32).astype(ml_dtypes.bfloat16)
    in_maps = []
    for core in range(N_CORES):
        b, th = core // 2, core % 2
        in_maps.append(
            {
                "q": np.ascontiguousarray(xb[b, :, 0:C]),
                "k": np.ascontiguousarray(xb[b, th * TL : (th + 1) * TL, C : 2 * C]),
                "v": np.ascontiguousarray(xb[b, :, 2 * C : 3 * C]),
            }
        )
    return in_maps


def kernel(x: np.ndarray) -> np.ndarray:
    global _NC_CACHE
    x = np.asarray(x, dtype=np.float32)
    assert x.shape == (B, T, 3 * C), x.shape

    if _NC_CACHE is None:
        _NC_CACHE = build_nc()
    nc = _NC_CACHE

    res = run_bass_kernel_spmd(nc, make_in_maps(x), core_ids=list(range(N_CORES)))

    out = np.empty((B, T, C), dtype=np.float32)
    for core in range(N_CORES):
        b, th = core // 2, core % 2
        out[b, th * TL : (th + 1) * TL] = res.results[core]["out"]
    return out



# revision 4
# speedup vs baseline: 1.0318x; 1.0318x over previous
"""Trainium2 Bass kernel for nn_AttLayer_9972914061697 (sparse_attention).

Reference computation (jax):
    q, k, v = split(x, 3, axis=-1)              # x: [B=4, T=4096, 3C=384]
    score   = einsum('btc,bsc->bts', k, q) / sqrt(C)
    out     = softmax(score, -1) @ v            # [B, T, C=128]

Sharding: 8 cores = 4 batches x 2 T-halves (data parallel, zero comm).
Each core holds full q, v of its batch plus its 2048-row k chunk and
produces its 2048-row output chunk. q and k are shipped host-transposed
([C, T] layout) so no XBAR-transpose DMAs are needed; all tensors bf16.

Per-core algorithm (v2):
  - S_T[s, t] = sum_c q[s,c] k[t,c] via qT_chunk.T @ kT, staged in PSUM as
    4 stages of [128, 512] per s-chunk (2 rotating bank tiles).
  - P_T = exp(S_T / sqrt(C)) split across two engines per chunk:
    stages 0,2 on ScalarE (table exp), stages 1,3 on VectorE via the
    Schraudolph bit-trick: bf16 bits of exp(x) ~= int16(x*A + B), computed
    as one tensor_scalar (mult, add) with int16 output, bitcast to bf16.
  - PV accumulates over ALL 32 s-chunks directly in PSUM: the 16 [128,129]
    output accumulators ([t-tile, v|rowsum]) are packed 3-per-bank at
    130-column stride in 6 bank tiles.  A zeroing matmul (start=True) per
    bank sets every element's has_written bit up front, so the 3
    interleaved accumulation chains per bank all run with start=False.
  - Tail: per bank, VectorE reciprocal of the 3 rowsum columns, then
    per-tile tensor_scalar/scalar-mul (split DVE/ACT) into an SBUF staging
    tile and DMA out.
"""

import numpy as np
import ml_dtypes

import concourse.bass as bass
import concourse.tile as tile
from concourse import bacc, mybir
from concourse.bass_utils import run_bass_kernel_spmd
from concourse.alu_op_type import AluOpType

F32 = mybir.dt.float32
BF16 = mybir.dt.bfloat16
I16 = mybir.dt.int16

B = 4
T = 4096
C = 128
N_CORES = 8
TL = T // 2            # 2048 t-rows per core
NSC = T // 128         # 32 s-chunks
NTT = TL // 128        # 16 t-tiles
STG = 512              # S staging width (one PSUM bank)
NSTG = TL // STG       # 4 stages per s-chunk
VW = 132               # v chunk pitch: 128 v cols + ones col + pad (8B align)

SCALE = 1.0 / float(np.sqrt(C))
LOG2E = float(np.log2(np.e))
SCH_A = float(np.sqrt(128.0)) * LOG2E          # x*A maps raw score to 128*log2(P)
SCH_B = 127.0 * 128.0 - 5.77                   # bias, calibrated for round-to-nearest

# out accumulator tt -> (bank, col offset); 3 tiles per bank, 130-col stride
OBANK = [(tt // 3, 130 * (tt % 3)) for tt in range(NTT)]
NBANK = 6


def build_nc():
    nc = bacc.Bacc()
    qT = nc.declare_dram_parameter("qT", [C, T], BF16, isOutput=False)
    kT = nc.declare_dram_parameter("kT", [C, TL], BF16, isOutput=False)
    v = nc.declare_dram_parameter("v", [T, C], BF16, isOutput=False)
    out = nc.declare_dram_parameter("out", [TL, C], F32, isOutput=True)

    vw = v[:].rearrange("(n p) c -> p n c", p=128)    # [128, 32, 128]
    ov = out[:].rearrange("(n p) c -> p n c", p=128)  # [128, 16, 128]

    with tile.TileContext(nc) as tc:
        with (
            tc.tile_pool(name="const", bufs=1) as const_pool,
            tc.tile_pool(name="qkt", bufs=1) as qkt_pool,
            tc.tile_pool(name="vbuf", bufs=1) as v_pool,
            tc.tile_pool(name="pT", bufs=4) as pT_pool,
            tc.tile_pool(name="ost", bufs=2) as ost_pool,
            tc.tile_pool(name="spsum", bufs=2, space="PSUM") as spsum,
            tc.tile_pool(name="opsum", bufs=1, space="PSUM") as opsum,
        ):
            qT_t = qkt_pool.tile([128, T], BF16, tag="qT")
            kT_t = qkt_pool.tile([128, TL], BF16, tag="kT")
            vv = v_pool.tile([128, NSC * VW], BF16)
            vv3 = vv[:].rearrange("p (n c) -> p n c", c=VW)
            zer = const_pool.tile([128, 390], BF16, tag="zer")
            rcp = const_pool.tile([128, NTT], F32, tag="rcp")

            nc.vector.memset(zer[:], 0.0)
            nc.vector.memset(vv3[:, :, C : C + 1], 1.0)

            # warm the ACT exp table so the ~2.7us table load overlaps
            # the prologue DMA instead of stalling the first real exp
            warm = const_pool.tile([128, 8], F32, tag="warm")
            nc.vector.memset(warm[:], 0.0)
            nc.scalar.activation(
                warm[:], warm[:], mybir.ActivationFunctionType.Exp, scale=1.0
            )

            # output accumulators: 6 PSUM bank tiles, 3 chains each
            obank = [
                opsum.tile([128, 390], F32, tag=f"ob{b_}", name=f"ob{b_}")
                for b_ in range(NBANK)
            ]
            # zeroing matmuls: set every element + its has_written bit so the
            # packed accumulation chains below can all run start=False
            for b_ in range(NBANK):
                nc.tensor.matmul(
                    obank[b_][:, 0:390], zer[:, 0:128], zer[:, 0:390],
                    start=True, stop=True,
                )

            # prologue DMA: kT whole, first qT pieces, first v piece
            nc.sync.dma_start(out=kT_t[:], in_=kT[:])

            def load_q(piece):  # 512 qT cols = 4 chunks
                nc.sync.dma_start(
                    out=qT_t[:, piece * 512 : (piece + 1) * 512],
                    in_=qT[:, piece * 512 : (piece + 1) * 512],
                )

            def load_v(piece):  # 4 v chunks
                nc.sync.dma_start(
                    out=vv3[:, piece * 4 : piece * 4 + 4, 0:C],
                    in_=vw[:, piece * 4 : piece * 4 + 4, :],
                )

            load_q(0)
            load_v(0)
            load_q(1)

            def qk_stage(j, pT_j, pT_j16, st):
                lhs = qT_t[:, j * 128 : (j + 1) * 128]
                s_st = spsum.tile([128, STG], F32, tag="S")
                nc.tensor.matmul(
                    s_st[:], lhs, kT_t[:, st * STG : (st + 1) * STG],
                    start=True, stop=True,
                )
                dst = slice(st * STG, (st + 1) * STG)
                if st % 2 == 0:
                    nc.scalar.activation(
                        pT_j[:, dst], s_st[:],
                        mybir.ActivationFunctionType.Exp, scale=SCALE,
                    )
                else:
                    nc.vector.tensor_scalar(
                        pT_j16[:, dst], s_st[:], SCH_A, SCH_B,
                        AluOpType.mult, AluOpType.add,
                    )

            def pv_tiles(j, pT_j, tts, final):
                for tt in tts:
                    b_, off = OBANK[tt]
                    nc.tensor.matmul(
                        obank[b_][:, off : off + 129],
                        pT_j[:, tt * 128 : (tt + 1) * 128],
                        vv3[:, j, 0 : C + 1],
                        start=False, stop=final, skip_group_check=True,
                    )
                    if final and (tt % 3 == 2 or tt == NTT - 1):
                        flush_bank(tt // 3)

            def flush_bank(b_):
                ntile = 3 if b_ < 5 else 1
                bank = obank[b_]
                ost = ost_pool.tile([128, 3 * 128], F32, tag="ost")
                ost3 = ost[:].rearrange("p (t c) -> p t c", c=128)
                for i in range(ntile):
                    tt = b_ * 3 + i
                    off = 130 * i
                    nc.vector.reciprocal(
                        rcp[:, tt : tt + 1], bank[:, off + 128 : off + 129]
                    )
                    if tt % 2 == 0:
                        nc.vector.tensor_scalar_mul(
                            ost[:, i * 128 : (i + 1) * 128],
                            bank[:, off : off + 128],
                            rcp[:, tt : tt + 1],
                        )
                    else:
                        nc.scalar.mul(
                            ost[:, i * 128 : (i + 1) * 128],
                            bank[:, off : off + 128],
                            rcp[:, tt : tt + 1],
                        )
                nc.sync.dma_start(
                    out=ov[:, b_ * 3 : b_ * 3 + ntile, :], in_=ost3[:, 0:ntile, :]
                )

            # ---- software-pipelined main loop ----
            # per chunk: QK stages 0,1 | PV(prev) tiles 0..7 | stages 2,3 |
            # PV(prev) tiles 8..15 — so ready PV work fills the PE while the
            # exp engines drain the S stages (S bufs rotate 2-deep).
            prev = None
            for j in range(NSC):
                if j % 4 == 0:
                    g = j // 4
                    if g + 2 < 8:
                        load_q(g + 2)
                    if g + 1 < 8:
                        load_v(g + 1)
                pT_j = pT_pool.tile([128, TL], BF16, tag="pT")
                pT_j16 = pT_j[:].bitcast(I16)
                qk_stage(j, pT_j, pT_j16, 0)
                qk_stage(j, pT_j, pT_j16, 1)
                if prev is not None:
                    pv_tiles(j - 1, prev, range(0, 8), final=False)
                qk_stage(j, pT_j, pT_j16, 2)
                qk_stage(j, pT_j, pT_j16, 3)
                if prev is not None:
                    pv_tiles(j - 1, prev, range(8, NTT), final=False)
                prev = pT_j
            pv_tiles(NSC - 1, prev, range(NTT), final=True)

    nc.finalize()
    return nc


_NC_CACHE = None


def make_in_maps(x: np.ndarray):
    xb = np.asarray(x, dtype=np.float32).astype(ml_dtypes.bfloat16)
    in_maps = []
    for core in range(N_CORES):
        b, th = core // 2, core % 2
        in_maps.append(
            {
                "qT": np.ascontiguousarray(xb[b, :, 0:C].T),
                "kT": np.ascontiguousarray(xb[b, th * TL : (th + 1) * TL, C : 2 * C].T),
                "v": np.ascontiguousarray(xb[b, :, 2 * C : 3 * C]),
            }
        )
    return in_maps


def kernel(x: np.ndarray) -> np.ndarray:
    global _NC_CACHE
    x = np.asarray(x, dtype=np.float32)
    assert x.shape == (B, T, 3 * C), x.shape

    if _NC_CACHE is None:
        _NC_CACHE = build_nc()
    nc = _NC_CACHE

    res = run_bass_kernel_spmd(nc, make_in_maps(x), core_ids=list(range(N_CORES)))

    out = np.empty((B, T, C), dtype=np.float32)
    for core in range(N_CORES):
        b, th = core // 2, core % 2
        out[b, th * TL : (th + 1) * TL] = res.results[core]["out"]
    return out


# revision 7
# speedup vs baseline: 1.3377x; 1.2965x over previous
"""Trainium2 Bass kernel for nn_AttLayer_9972914061697 (sparse_attention).

Reference computation (jax):
    q, k, v = split(x, 3, axis=-1)              # x: [B=4, T=4096, 3C=384]
    score   = einsum('btc,bsc->bts', k, q) / sqrt(C)
    out     = softmax(score, -1) @ v            # [B, T, C=128]

Sharding: 8 cores = 4 batches x 2 T-halves (data parallel, zero comm).
Each core holds full q, v of its batch plus its 2048-row k chunk and
produces its 2048-row output chunk. q and k are shipped host-transposed
([C, T] layout) so no XBAR-transpose DMAs are needed; all tensors bf16.

Per-core algorithm (v2):
  - S_T[s, t] = sum_c q[s,c] k[t,c] via qT_chunk.T @ kT, staged in PSUM as
    4 stages of [128, 512] per s-chunk (2 rotating bank tiles).
  - P_T = exp(S_T / sqrt(C)) split across two engines per chunk:
    stages 0,2 on ScalarE (table exp), stages 1,3 on VectorE via the
    Schraudolph bit-trick: bf16 bits of exp(x) ~= int16(x*A + B), computed
    as one tensor_scalar (mult, add) with int16 output, bitcast to bf16.
  - PV accumulates over ALL 32 s-chunks directly in PSUM: the 16 [128,129]
    output accumulators ([t-tile, v|rowsum]) are packed 3-per-bank at
    130-column stride in 6 bank tiles.  A zeroing matmul (start=True) per
    bank sets every element's has_written bit up front, so the 3
    interleaved accumulation chains per bank all run with start=False.
  - Tail: per bank, VectorE reciprocal of the 3 rowsum columns, then
    per-tile tensor_scalar/scalar-mul (split DVE/ACT) into an SBUF staging
    tile and DMA out.
"""

import numpy as np
import ml_dtypes

import concourse.bass as bass
import concourse.tile as tile
from concourse import bacc, mybir
from concourse.bass_utils import run_bass_kernel_spmd
from concourse.alu_op_type import AluOpType

F32 = mybir.dt.float32
BF16 = mybir.dt.bfloat16
I16 = mybir.dt.int16

B = 4
T = 4096
C = 128
N_CORES = 8
TL = T // 2            # 2048 t-rows per core
NSC = T // 128         # 32 s-chunks
NTT = TL // 128        # 16 t-tiles
STG = 512              # S staging width (one PSUM bank)
NSTG = TL // STG       # 4 stages per s-chunk
VW = 132               # v chunk pitch: 128 v cols + ones col + pad (8B align)

SCALE = 1.0 / float(np.sqrt(C))
LOG2E = float(np.log2(np.e))
SCH_A = float(np.sqrt(128.0)) * LOG2E          # x*A maps raw score to 128*log2(P)
SCH_B = 127.0 * 128.0 - 5.77                   # bias, calibrated for round-to-nearest

# per-pass out accumulators: 8 t-tiles -> 3 banks, 3 tiles per bank at
# 130-col stride (last bank holds 2)
OBANK = [(lt // 3, 130 * (lt % 3)) for lt in range(8)]
NBANK = 3
BANKW = [390, 390, 260]
BTILES = [3, 3, 2]


def build_nc():
    nc = bacc.Bacc()
    qT = nc.declare_dram_parameter("qT", [C, T], BF16, isOutput=False)
    kT = nc.declare_dram_parameter("kT", [C, TL], BF16, isOutput=False)
    v = nc.declare_dram_parameter("v", [T, C], BF16, isOutput=False)
    out = nc.declare_dram_parameter("out", [TL, C], F32, isOutput=True)

    vw = v[:].rearrange("(n p) c -> p n c", p=128)    # [128, 32, 128]
    ov = out[:].rearrange("(n p) c -> p n c", p=128)  # [128, 16, 128]

    with tile.TileContext(nc) as tc:
        with (
            tc.tile_pool(name="const", bufs=1) as const_pool,
            tc.tile_pool(name="qkt", bufs=1) as qkt_pool,
            tc.tile_pool(name="vbuf", bufs=1) as v_pool,
            tc.tile_pool(name="pT", bufs=4) as pT_pool,
            tc.tile_pool(name="ost", bufs=4) as ost_pool,
            tc.tile_pool(name="spsum", bufs=5, space="PSUM") as spsum,
            tc.tile_pool(name="opsum", bufs=1, space="PSUM") as opsum,
        ):
            qT_t = qkt_pool.tile([128, T], BF16, tag="qT")
            kT_t = qkt_pool.tile([128, TL], BF16, tag="kT")
            vv = v_pool.tile([128, NSC * VW], BF16)
            vv3 = vv[:].rearrange("p (n c) -> p n c", c=VW)
            zer = const_pool.tile([128, 390], BF16, tag="zer")
            rcp = const_pool.tile([128, NTT], F32, tag="rcp")

            nc.vector.memset(zer[:], 0.0)
            nc.vector.memset(vv3[:, :, C : C + 1], 1.0)

            # warm the ACT exp table so the ~2.7us table load overlaps
            # the prologue DMA instead of stalling the first real exp
            warm = const_pool.tile([128, 8], F32, tag="warm")
            nc.vector.memset(warm[:], 0.0)
            nc.scalar.activation(
                warm[:], warm[:], mybir.ActivationFunctionType.Exp, scale=1.0
            )

            # prologue DMA: kT whole on sync queue, qT pieces on sync,
            # v pieces on the (otherwise idle) gpsimd queue
            nc.sync.dma_start(out=kT_t[:], in_=kT[:])

            def load_q(piece):  # 512 qT cols = 4 chunks
                nc.sync.dma_start(
                    out=qT_t[:, piece * 512 : (piece + 1) * 512],
                    in_=qT[:, piece * 512 : (piece + 1) * 512],
                )

            def load_v(piece):  # 4 v chunks
                nc.gpsimd.dma_start(
                    out=vv3[:, piece * 4 : piece * 4 + 4, 0:C],
                    in_=vw[:, piece * 4 : piece * 4 + 4, :],
                )

            load_q(0)
            load_v(0)
            load_q(1)
            load_v(1)
            load_q(2)

            def qk_stage(ph, j, pT_j, pT_j16, st):
                # st is pass-local (0,1); global t-stage is 2*ph+st
                gst = 2 * ph + st
                lhs = qT_t[:, j * 128 : (j + 1) * 128]
                s_st = spsum.tile([128, STG], F32, tag="S")
                nc.tensor.matmul(
                    s_st[:], lhs, kT_t[:, gst * STG : (gst + 1) * STG],
                    start=True, stop=True,
                )
                dst = slice(st * STG, (st + 1) * STG)
                if st % 2 == 0:
                    nc.scalar.activation(
                        pT_j[:, dst], s_st[:],
                        mybir.ActivationFunctionType.Exp, scale=SCALE,
                    )
                else:
                    nc.vector.tensor_scalar(
                        pT_j16[:, dst], s_st[:], SCH_A, SCH_B,
                        AluOpType.mult, AluOpType.add,
                    )

            def pv_tiles(obank, ph, j, pT_j, final):
                for lt in range(8):
                    b_, off = OBANK[lt]
                    nc.tensor.matmul(
                        obank[b_][:, off : off + 129],
                        pT_j[:, lt * 128 : (lt + 1) * 128],
                        vv3[:, j, 0 : C + 1],
                        start=False, stop=final, skip_group_check=True,
                    )
                    if final and lt + 1 in (3, 6, 8):
                        flush_bank(obank, ph, lt // 3)

            def flush_bank(obank, ph, b_):
                ntile = BTILES[b_]
                bank = obank[b_]
                ost = ost_pool.tile([128, 3 * 128], F32, tag="ost")
                ost3 = ost[:].rearrange("p (t c) -> p t c", c=128)
                for i in range(ntile):
                    tt = 8 * ph + b_ * 3 + i
                    off = 130 * i
                    nc.vector.reciprocal(
                        rcp[:, tt : tt + 1], bank[:, off + 128 : off + 129]
                    )
                    if tt % 2 == 0:
                        nc.vector.tensor_scalar_mul(
                            ost[:, i * 128 : (i + 1) * 128],
                            bank[:, off : off + 128],
                            rcp[:, tt : tt + 1],
                        )
                    else:
                        nc.scalar.mul(
                            ost[:, i * 128 : (i + 1) * 128],
                            bank[:, off : off + 128],
                            rcp[:, tt : tt + 1],
                        )
                nc.gpsimd.dma_start(
                    out=ov[:, 8 * ph + b_ * 3 : 8 * ph + b_ * 3 + ntile, :],
                    in_=ost3[:, 0:ntile, :],
                )

            # ---- two t-half passes over all 32 s-chunks ----
            # pass ph covers t-cols [ph*1024, (ph+1)*1024) = out tiles 8ph..8ph+7.
            # Only 3 PSUM banks of accumulators per pass, so S staging gets 5
            # rotating banks and the QK->exp->QK WAR chain never binds.
            # Pass A's flush overlaps pass B's compute.
            for ph in range(2):
                obank = [
                    opsum.tile([128, BANKW[b_]], F32, tag=f"ob{b_}", name=f"ob{b_}")
                    for b_ in range(NBANK)
                ]
                # zeroing matmuls set every element + its has_written bit so
                # the 3 packed chains per bank all run start=False
                for b_ in range(NBANK):
                    nc.tensor.matmul(
                        obank[b_][:, 0 : BANKW[b_]], zer[:, 0:128],
                        zer[:, 0 : BANKW[b_]], start=True, stop=True,
                    )
                prev = None
                for j in range(NSC):
                    if ph == 0 and j % 4 == 0:
                        g = j // 4
                        if g + 3 < 8:
                            load_q(g + 3)
                        if g + 2 < 8:
                            load_v(g + 2)
                    pT_j = pT_pool.tile([128, 2 * STG], BF16, tag="pT")
                    pT_j16 = pT_j[:].bitcast(I16)
                    qk_stage(ph, j, pT_j, pT_j16, 0)
                    qk_stage(ph, j, pT_j, pT_j16, 1)
                    if prev is not None:
                        pv_tiles(obank, ph, j - 1, prev, final=False)
                    prev = pT_j
                pv_tiles(obank, ph, NSC - 1, prev, final=True)

    nc.finalize()
    return nc


_NC_CACHE = None


def make_in_maps(x: np.ndarray):
    xb = np.asarray(x, dtype=np.float32).astype(ml_dtypes.bfloat16)
    in_maps = []
    for core in range(N_CORES):
        b, th = core // 2, core % 2
        in_maps.append(
            {
                "qT": np.ascontiguousarray(xb[b, :, 0:C].T),
                "kT": np.ascontiguousarray(xb[b, th * TL : (th + 1) * TL, C : 2 * C].T),
                "v": np.ascontiguousarray(xb[b, :, 2 * C : 3 * C]),
            }
        )
    return in_maps


def kernel(x: np.ndarray) -> np.ndarray:
    global _NC_CACHE
    x = np.asarray(x, dtype=np.float32)
    assert x.shape == (B, T, 3 * C), x.shape

    if _NC_CACHE is None:
        _NC_CACHE = build_nc()
    nc = _NC_CACHE

    res = run_bass_kernel_spmd(nc, make_in_maps(x), core_ids=list(range(N_CORES)))

    out = np.empty((B, T, C), dtype=np.float32)
    for core in range(N_CORES):
        b, th = core // 2, core % 2
        out[b, th * TL : (th + 1) * TL] = res.results[core]["out"]
    return out


# revision 10
# speedup vs baseline: 1.3472x; 1.0071x over previous
"""Trainium2 Bass kernel for nn_AttLayer_9972914061697 (sparse_attention).

Reference computation (jax):
    q, k, v = split(x, 3, axis=-1)              # x: [B=4, T=4096, 3C=384]
    score   = einsum('btc,bsc->bts', k, q) / sqrt(C)
    out     = softmax(score, -1) @ v            # [B, T, C=128]

Sharding: 8 cores = 4 batches x 2 T-halves (data parallel, zero comm).
Each core holds full q, v of its batch plus its 2048-row k chunk and
produces its 2048-row output chunk. q and k are shipped host-transposed
([C, T] layout) so no XBAR-transpose DMAs are needed; all tensors bf16.

Per-core algorithm (v2):
  - S_T[s, t] = sum_c q[s,c] k[t,c] via qT_chunk.T @ kT, staged in PSUM as
    4 stages of [128, 512] per s-chunk (2 rotating bank tiles).
  - P_T = exp(S_T / sqrt(C)) split across two engines per chunk:
    stages 0,2 on ScalarE (table exp), stages 1,3 on VectorE via the
    Schraudolph bit-trick: bf16 bits of exp(x) ~= int16(x*A + B), computed
    as one tensor_scalar (mult, add) with int16 output, bitcast to bf16.
  - PV accumulates over ALL 32 s-chunks directly in PSUM: the 16 [128,129]
    output accumulators ([t-tile, v|rowsum]) are packed 3-per-bank at
    130-column stride in 6 bank tiles.  A zeroing matmul (start=True) per
    bank sets every element's has_written bit up front, so the 3
    interleaved accumulation chains per bank all run with start=False.
  - Tail: per bank, VectorE reciprocal of the 3 rowsum columns, then
    per-tile tensor_scalar/scalar-mul (split DVE/ACT) into an SBUF staging
    tile and DMA out.
"""

import numpy as np
import ml_dtypes

import concourse.bass as bass
import concourse.tile as tile
from concourse import bacc, mybir
from concourse.bass_utils import run_bass_kernel_spmd
from concourse.alu_op_type import AluOpType

F32 = mybir.dt.float32
BF16 = mybir.dt.bfloat16
I16 = mybir.dt.int16

B = 4
T = 4096
C = 128
N_CORES = 8
TL = T // 2            # 2048 t-rows per core
NSC = T // 128         # 32 s-chunks
NTT = TL // 128        # 16 t-tiles
STG = 512              # S staging width (one PSUM bank)
NSTG = TL // STG       # 4 stages per s-chunk
VW = 132               # v chunk pitch: 128 v cols + ones col + pad (8B align)

SCALE = 1.0 / float(np.sqrt(C))
LOG2E = float(np.log2(np.e))
SCH_A = float(np.sqrt(128.0)) * LOG2E          # x*A maps raw score to 128*log2(P)
SCH_B = 127.0 * 128.0 - 5.77                   # bias, calibrated for round-to-nearest

# per-pass out accumulators: 8 t-tiles -> 3 banks, 3 tiles per bank at
# 130-col stride (last bank holds 2)
OBANK = [(lt // 3, 130 * (lt % 3)) for lt in range(8)]
NBANK = 3
BANKW = [390, 390, 260]
BTILES = [3, 3, 2]


def build_nc():
    nc = bacc.Bacc()
    qT = nc.declare_dram_parameter("qT", [C, T], BF16, isOutput=False)
    kT = nc.declare_dram_parameter("kT", [C, TL], BF16, isOutput=False)
    v = nc.declare_dram_parameter("v", [T, C], BF16, isOutput=False)
    out = nc.declare_dram_parameter("out", [TL, C], F32, isOutput=True)

    vw = v[:].rearrange("(n p) c -> p n c", p=128)    # [128, 32, 128]
    ov = out[:].rearrange("(n p) c -> p n c", p=128)  # [128, 16, 128]

    with tile.TileContext(nc) as tc:
        with (
            tc.tile_pool(name="const", bufs=1) as const_pool,
            tc.tile_pool(name="qkt", bufs=1) as qkt_pool,
            tc.tile_pool(name="vbuf", bufs=1) as v_pool,
            tc.tile_pool(name="pT", bufs=4) as pT_pool,
            tc.tile_pool(name="ost", bufs=4) as ost_pool,
            tc.tile_pool(name="spsum", bufs=5, space="PSUM") as spsum,
            tc.tile_pool(name="opsum", bufs=1, space="PSUM") as opsum,
        ):
            qT_t = qkt_pool.tile([128, T], BF16, tag="qT")
            kT_t = qkt_pool.tile([128, TL], BF16, tag="kT")
            vv = v_pool.tile([128, NSC * VW], BF16)
            vv3 = vv[:].rearrange("p (n c) -> p n c", c=VW)
            zer = const_pool.tile([128, 390], BF16, tag="zer")
            rcp = const_pool.tile([128, NTT], F32, tag="rcp")

            nc.vector.memset(zer[:], 0.0)
            nc.vector.memset(vv3[:, :, C : C + 1], 1.0)

            # warm the ACT exp table so the ~2.7us table load overlaps
            # the prologue DMA instead of stalling the first real exp
            warm = const_pool.tile([128, 8], F32, tag="warm")
            nc.vector.memset(warm[:], 0.0)
            nc.scalar.activation(
                warm[:], warm[:], mybir.ActivationFunctionType.Exp, scale=1.0
            )

            def load_q(piece):  # 512 qT cols = 4 chunks
                nc.sync.dma_start(
                    out=qT_t[:, piece * 512 : (piece + 1) * 512],
                    in_=qT[:, piece * 512 : (piece + 1) * 512],
                )

            def load_v(piece):  # 4 v chunks
                nc.sync.dma_start(
                    out=vv3[:, piece * 4 : piece * 4 + 4, 0:C],
                    in_=vw[:, piece * 4 : piece * 4 + 4, :],
                )

            # prologue DMA, all on the sync HWDGE queue, urgency order:
            # first kT half + first qT piece unblock pass A's QK immediately
            nc.sync.dma_start(out=kT_t[:, 0:1024], in_=kT[:, 0:1024])
            load_q(0)
            load_v(0)
            load_q(1)
            nc.sync.dma_start(out=kT_t[:, 1024:2048], in_=kT[:, 1024:2048])
            load_v(1)
            load_q(2)

            def qk_stage(ph, j, pT_j, pT_j16, st):
                # st is pass-local (0,1); global t-stage is 2*ph+st
                gst = 2 * ph + st
                lhs = qT_t[:, j * 128 : (j + 1) * 128]
                s_st = spsum.tile([128, STG], F32, tag="S")
                nc.tensor.matmul(
                    s_st[:], lhs, kT_t[:, gst * STG : (gst + 1) * STG],
                    start=True, stop=True,
                )
                dst = slice(st * STG, (st + 1) * STG)
                if st % 2 == 0:
                    nc.scalar.activation(
                        pT_j[:, dst], s_st[:],
                        mybir.ActivationFunctionType.Exp, scale=SCALE,
                    )
                else:
                    nc.vector.tensor_scalar(
                        pT_j16[:, dst], s_st[:], SCH_A, SCH_B,
                        AluOpType.mult, AluOpType.add,
                    )

            def pv_tiles(obank, ph, j, pT_j, final):
                for lt in range(8):
                    b_, off = OBANK[lt]
                    nc.tensor.matmul(
                        obank[b_][:, off : off + 129],
                        pT_j[:, lt * 128 : (lt + 1) * 128],
                        vv3[:, j, 0 : C + 1],
                        start=False, stop=final, skip_group_check=True,
                    )
                    if final and lt + 1 in (3, 6, 8):
                        flush_bank(obank, ph, lt // 3)

            def flush_bank(obank, ph, b_):
                ntile = BTILES[b_]
                bank = obank[b_]
                ost = ost_pool.tile([128, 3 * 128], F32, tag="ost")
                ost3 = ost[:].rearrange("p (t c) -> p t c", c=128)
                for i in range(ntile):
                    tt = 8 * ph + b_ * 3 + i
                    off = 130 * i
                    nc.vector.reciprocal(
                        rcp[:, tt : tt + 1], bank[:, off + 128 : off + 129]
                    )
                    if tt % 2 == 0:
                        nc.vector.tensor_scalar_mul(
                            ost[:, i * 128 : (i + 1) * 128],
                            bank[:, off : off + 128],
                            rcp[:, tt : tt + 1],
                        )
                    else:
                        nc.scalar.mul(
                            ost[:, i * 128 : (i + 1) * 128],
                            bank[:, off : off + 128],
                            rcp[:, tt : tt + 1],
                        )
                nc.sync.dma_start(
                    out=ov[:, 8 * ph + b_ * 3 : 8 * ph + b_ * 3 + ntile, :],
                    in_=ost3[:, 0:ntile, :],
                )

            # ---- two t-half passes over all 32 s-chunks ----
            # pass ph covers t-cols [ph*1024, (ph+1)*1024) = out tiles 8ph..8ph+7.
            # Only 3 PSUM banks of accumulators per pass, so S staging gets 5
            # rotating banks and the QK->exp->QK WAR chain never binds.
            # Pass A's flush overlaps pass B's compute.
            for ph in range(2):
                obank = [
                    opsum.tile([128, BANKW[b_]], F32, tag=f"ob{b_}", name=f"ob{b_}")
                    for b_ in range(NBANK)
                ]
                # zeroing matmuls set every element + its has_written bit so
                # the 3 packed chains per bank all run start=False
                for b_ in range(NBANK):
                    nc.tensor.matmul(
                        obank[b_][:, 0 : BANKW[b_]], zer[:, 0:128],
                        zer[:, 0 : BANKW[b_]], start=True, stop=True,
                    )
                # PV runs TWO chunks behind QK/exp so the PV LDWEIGHTS never
                # waits on a just-finished exp
                prevs = []
                for j in range(NSC):
                    if ph == 0 and j % 4 == 0:
                        g = j // 4
                        if g + 3 < 8:
                            load_q(g + 3)
                        if g + 2 < 8:
                            load_v(g + 2)
                    pT_j = pT_pool.tile([128, 2 * STG], BF16, tag="pT")
                    pT_j16 = pT_j[:].bitcast(I16)
                    qk_stage(ph, j, pT_j, pT_j16, 0)
                    qk_stage(ph, j, pT_j, pT_j16, 1)
                    prevs.append(pT_j)
                    if j >= 2:
                        pv_tiles(obank, ph, j - 2, prevs[j - 2], final=False)
                pv_tiles(obank, ph, NSC - 2, prevs[NSC - 2], final=False)
                pv_tiles(obank, ph, NSC - 1, prevs[NSC - 1], final=True)

    nc.finalize()
    return nc


_NC_CACHE = None


def make_in_maps(x: np.ndarray):
    xb = np.asarray(x, dtype=np.float32).astype(ml_dtypes.bfloat16)
    in_maps = []
    for core in range(N_CORES):
        b, th = core // 2, core % 2
        in_maps.append(
            {
                "qT": np.ascontiguousarray(xb[b, :, 0:C].T),
                "kT": np.ascontiguousarray(xb[b, th * TL : (th + 1) * TL, C : 2 * C].T),
                "v": np.ascontiguousarray(xb[b, :, 2 * C : 3 * C]),
            }
        )
    return in_maps


def kernel(x: np.ndarray) -> np.ndarray:
    global _NC_CACHE
    x = np.asarray(x, dtype=np.float32)
    assert x.shape == (B, T, 3 * C), x.shape

    if _NC_CACHE is None:
        _NC_CACHE = build_nc()
    nc = _NC_CACHE

    res = run_bass_kernel_spmd(nc, make_in_maps(x), core_ids=list(range(N_CORES)))

    out = np.empty((B, T, C), dtype=np.float32)
    for core in range(N_CORES):
        b, th = core // 2, core % 2
        out[b, th * TL : (th + 1) * TL] = res.results[core]["out"]
    return out


# revision 13
# speedup vs baseline: 1.3974x; 1.0372x over previous
"""Trainium2 Bass kernel for nn_AttLayer_9972914061697 (sparse_attention).

Reference computation (jax):
    q, k, v = split(x, 3, axis=-1)              # x: [B=4, T=4096, 3C=384]
    score   = einsum('btc,bsc->bts', k, q) / sqrt(C)
    out     = softmax(score, -1) @ v            # [B, T, C=128]

Sharding: 8 cores = 4 batches x 2 T-halves (data parallel, zero comm).
Each core holds full q, v of its batch plus its 2048-row k chunk and
produces its 2048-row output chunk. q and k are shipped host-transposed
([C, T] layout) so no XBAR-transpose DMAs are needed; all tensors bf16.

Per-core algorithm (v2):
  - S_T[s, t] = sum_c q[s,c] k[t,c] via qT_chunk.T @ kT, staged in PSUM as
    4 stages of [128, 512] per s-chunk (2 rotating bank tiles).
  - P_T = exp(S_T / sqrt(C)) split across two engines per chunk:
    stages 0,2 on ScalarE (table exp), stages 1,3 on VectorE via the
    Schraudolph bit-trick: bf16 bits of exp(x) ~= int16(x*A + B), computed
    as one tensor_scalar (mult, add) with int16 output, bitcast to bf16.
  - PV accumulates over ALL 32 s-chunks directly in PSUM: the 16 [128,129]
    output accumulators ([t-tile, v|rowsum]) are packed 3-per-bank at
    130-column stride in 6 bank tiles.  A zeroing matmul (start=True) per
    bank sets every element's has_written bit up front, so the 3
    interleaved accumulation chains per bank all run with start=False.
  - Tail: per bank, VectorE reciprocal of the 3 rowsum columns, then
    per-tile tensor_scalar/scalar-mul (split DVE/ACT) into an SBUF staging
    tile and DMA out.
"""

import numpy as np
import ml_dtypes

import concourse.bass as bass
import concourse.tile as tile
from concourse import bacc, mybir
from concourse.bass_utils import run_bass_kernel_spmd
from concourse.alu_op_type import AluOpType

F32 = mybir.dt.float32
BF16 = mybir.dt.bfloat16
I16 = mybir.dt.int16

B = 4
T = 4096
C = 128
N_CORES = 8
TL = T // 2            # 2048 t-rows per core
NSC = T // 128         # 32 s-chunks
NTT = TL // 128        # 16 t-tiles
STG = 512              # S staging width (one PSUM bank)
NSTG = TL // STG       # 4 stages per s-chunk
VW = 132               # v chunk pitch: 128 v cols + ones col + pad (8B align)

SCALE = 1.0 / float(np.sqrt(C))
LOG2E = float(np.log2(np.e))
SCH_A = float(np.sqrt(128.0)) * LOG2E          # x*A maps raw score to 128*log2(P)
SCH_B = 127.0 * 128.0 - 5.77                   # bias, calibrated for round-to-nearest

# per-pass out accumulators: 8 t-tiles -> 3 banks, 3 tiles per bank at
# 130-col stride (last bank holds 2)
OBANK = [(lt // 3, 130 * (lt % 3)) for lt in range(8)]
NBANK = 3
BANKW = [390, 390, 260]
BTILES = [3, 3, 2]


def build_nc():
    nc = bacc.Bacc()
    qT = nc.declare_dram_parameter("qT", [C, T], BF16, isOutput=False)
    kT = nc.declare_dram_parameter("kT", [C, TL], BF16, isOutput=False)
    v = nc.declare_dram_parameter("v", [T, C], BF16, isOutput=False)
    out = nc.declare_dram_parameter("out", [TL, C], F32, isOutput=True)

    vw = v[:].rearrange("(n p) c -> p n c", p=128)    # [128, 32, 128]
    ov = out[:].rearrange("(n p) c -> p n c", p=128)  # [128, 16, 128]

    with tile.TileContext(nc) as tc:
        with (
            tc.tile_pool(name="const", bufs=1) as const_pool,
            tc.tile_pool(name="qkt", bufs=1) as qkt_pool,
            tc.tile_pool(name="vbuf", bufs=1) as v_pool,
            tc.tile_pool(name="pT", bufs=4) as pT_pool,
            tc.tile_pool(name="ost", bufs=4) as ost_pool,
            tc.tile_pool(name="spsum", bufs=5, space="PSUM") as spsum,
            tc.tile_pool(name="opsum", bufs=1, space="PSUM") as opsum,
        ):
            qT_t = qkt_pool.tile([128, T], BF16, tag="qT")
            kT_t = qkt_pool.tile([128, TL], BF16, tag="kT")
            vv = v_pool.tile([128, NSC * VW], BF16)
            vv3 = vv[:].rearrange("p (n c) -> p n c", c=VW)
            zer = const_pool.tile([128, 390], BF16, tag="zer")
            rcp = const_pool.tile([128, NTT], F32, tag="rcp")

            nc.vector.memset(zer[:], 0.0)
            nc.vector.memset(vv3[:, :, C : C + 1], 1.0)

            # warm the ACT exp table so the ~2.7us table load overlaps
            # the prologue DMA instead of stalling the first real exp
            warm = const_pool.tile([128, 8], F32, tag="warm")
            nc.vector.memset(warm[:], 0.0)
            nc.scalar.activation(
                warm[:], warm[:], mybir.ActivationFunctionType.Exp, scale=1.0
            )

            def load_q(piece):  # 512 qT cols = 4 chunks
                nc.sync.dma_start(
                    out=qT_t[:, piece * 512 : (piece + 1) * 512],
                    in_=qT[:, piece * 512 : (piece + 1) * 512],
                )

            def load_v(piece):  # 4 v chunks
                nc.sync.dma_start(
                    out=vv3[:, piece * 4 : piece * 4 + 4, 0:C],
                    in_=vw[:, piece * 4 : piece * 4 + 4, :],
                )

            # prologue DMA, all on the sync HWDGE queue, urgency order:
            # first kT half + first qT piece unblock pass A's QK immediately
            nc.sync.dma_start(out=kT_t[:, 0:1024], in_=kT[:, 0:1024])
            load_q(0)
            load_v(0)
            load_q(1)
            nc.sync.dma_start(out=kT_t[:, 1024:2048], in_=kT[:, 1024:2048])
            load_v(1)
            load_q(2)

            def qk_stage(ph, j, pT_j, pT_j16, st):
                # st is pass-local (0,1); global t-stage is 2*ph+st
                gst = 2 * ph + st
                lhs = qT_t[:, j * 128 : (j + 1) * 128]
                s_st = spsum.tile([128, STG], F32, tag="S")
                nc.tensor.matmul(
                    s_st[:], lhs, kT_t[:, gst * STG : (gst + 1) * STG],
                    start=True, stop=True,
                )
                dst = slice(st * STG, (st + 1) * STG)
                if st % 2 == 0:
                    nc.scalar.activation(
                        pT_j[:, dst], s_st[:],
                        mybir.ActivationFunctionType.Exp, scale=SCALE,
                    )
                else:
                    nc.vector.tensor_scalar(
                        pT_j16[:, dst], s_st[:], SCH_A, SCH_B,
                        AluOpType.mult, AluOpType.add,
                    )

            def pv_tiles(obank, ph, j, pT_j, final, flush_inline=False):
                for lt in range(8):
                    b_, off = OBANK[lt]
                    nc.tensor.matmul(
                        obank[b_][:, off : off + 129],
                        pT_j[:, lt * 128 : (lt + 1) * 128],
                        vv3[:, j, 0 : C + 1],
                        start=False, stop=final, skip_group_check=True,
                    )
                    if flush_inline and lt + 1 in (3, 6, 8):
                        flush_bank(obank, ph, lt // 3, spread_q=True)

            rcp3 = rcp[:].rearrange("p (t o) -> p t o", o=1)

            def flush_bank(obank, ph, b_, spread_q=False):
                ntile = BTILES[b_]
                bank = obank[b_]
                b3 = bank[:].rearrange("p (t x) -> p t x", x=130)
                t0 = 8 * ph + b_ * 3
                nc.vector.reciprocal(
                    rcp3[:, t0 : t0 + ntile, :], b3[:, 0:ntile, 128:129]
                )
                ost = ost_pool.tile([128, 3 * 128], F32, tag="ost")
                ost3 = ost[:].rearrange("p (t c) -> p t c", c=128)
                for i in range(ntile):
                    tt = t0 + i
                    off = 130 * i
                    if tt % 2 == 0:
                        nc.vector.tensor_scalar_mul(
                            ost[:, i * 128 : (i + 1) * 128],
                            bank[:, off : off + 128],
                            rcp[:, tt : tt + 1],
                        )
                    else:
                        nc.scalar.mul(
                            ost[:, i * 128 : (i + 1) * 128],
                            bank[:, off : off + 128],
                            rcp[:, tt : tt + 1],
                        )
                eng = [nc.sync, nc.scalar, nc.sync][b_] if spread_q else nc.sync
                eng.dma_start(
                    out=ov[:, t0 : t0 + ntile, :], in_=ost3[:, 0:ntile, :]
                )

            # ---- two t-half passes over all 32 s-chunks ----
            # pass ph covers t-cols [ph*1024, (ph+1)*1024) = out tiles 8ph..8ph+7.
            # Only 3 PSUM banks of accumulators per pass, so S staging gets 5
            # rotating banks and the QK->exp->QK WAR chain never binds.
            # Pass A's flush overlaps pass B's compute.
            def zero_mm(bank, width):
                # start=True writes zeros + sets every element's has_written
                # bit so the 3 packed chains per bank all run start=False
                nc.tensor.matmul(
                    bank[:, 0:width], zer[:, 0:128], zer[:, 0:width],
                    start=True, stop=True,
                )

            pending = None  # pass A's (obank, ph) awaiting flush during pass B
            for ph in range(2):
                obank = [
                    opsum.tile([128, BANKW[b_]], F32, tag=f"ob{b_}", name=f"ob{b_}")
                    for b_ in range(NBANK)
                ]
                if pending is None:
                    for b_ in range(NBANK):
                        zero_mm(obank[b_], BANKW[b_])
                # PV runs TWO chunks behind QK/exp so the PV LDWEIGHTS never
                # waits on a just-finished exp
                prevs = []
                for j in range(NSC):
                    if ph == 0 and j % 4 == 0:
                        g = j // 4
                        if g + 3 < 8:
                            load_q(g + 3)
                        if g + 2 < 8:
                            load_v(g + 2)
                    pT_j = pT_pool.tile([128, 2 * STG], BF16, tag="pT")
                    pT_j16 = pT_j[:].bitcast(I16)
                    qk_stage(ph, j, pT_j, pT_j16, 0)
                    qk_stage(ph, j, pT_j, pT_j16, 1)
                    prevs.append(pT_j)
                    if pending is not None and j < NBANK:
                        # spread the previous pass's flush + this pass's
                        # zeroing over the first chunks (one bank per chunk)
                        # so the exp engines never see a burst of flush work
                        flush_bank(*pending, j)
                        zero_mm(obank[j], BANKW[j])
                    if j >= 2:
                        pv_tiles(obank, ph, j - 2, prevs[j - 2], final=False)
                pv_tiles(obank, ph, NSC - 2, prevs[NSC - 2], final=False)
                pv_tiles(
                    obank, ph, NSC - 1, prevs[NSC - 1],
                    final=True, flush_inline=(ph == 1),
                )
                pending = (obank, ph)

    nc.finalize()
    return nc


_NC_CACHE = None


def make_in_maps(x: np.ndarray):
    xb = np.asarray(x, dtype=np.float32).astype(ml_dtypes.bfloat16)
    in_maps = []
    for core in range(N_CORES):
        b, th = core // 2, core % 2
        in_maps.append(
            {
                "qT": np.ascontiguousarray(xb[b, :, 0:C].T),
                "kT": np.ascontiguousarray(xb[b, th * TL : (th + 1) * TL, C : 2 * C].T),
                "v": np.ascontiguousarray(xb[b, :, 2 * C : 3 * C]),
            }
        )
    return in_maps


def kernel(x: np.ndarray) -> np.ndarray:
    global _NC_CACHE
    x = np.asarray(x, dtype=np.float32)
    assert x.shape == (B, T, 3 * C), x.shape

    if _NC_CACHE is None:
        _NC_CACHE = build_nc()
    nc = _NC_CACHE

    res = run_bass_kernel_spmd(nc, make_in_maps(x), core_ids=list(range(N_CORES)))

    out = np.empty((B, T, C), dtype=np.float32)
    for core in range(N_CORES):
        b, th = core // 2, core % 2
        out[b, th * TL : (th + 1) * TL] = res.results[core]["out"]
    return out
